# revision 3
# baseline (speedup 1.0000x reference)
"""Bass/Trainium2 kernel for nn_Channel_attention (bottom-16 channel gather).

reference semantics (per sample b):
    weight = mean(x[b], axis=(H, W))           # [C]
    idx    = argsort(weight)[:16]              # ascending pooled value
    out[b] = x[b, idx]                         # [16, H, W]

Strategy: pure data parallel, B=16 sharded 2 samples per core over 8 cores.
Per core (x shard viewed as [512, 16384] = [(sample, channel), H*W]):

Sample 0 (fully overlapped under sample 1's loads):
  Stream [128ch, 2048] tiles through a small pool, DVE reduce -> per-channel
  sums, two rounds of max8/max_index/match_replace -> bottom-16 indices,
  SWDGE indirect-gather the selected channels from HBM, store to y0.

Sample 1 (the critical tail):
  Loads land in a persistent 16 MiB SBUF tile (128 part x 2 halves x 16 KiB),
  reduced as they arrive.  Once all 256 sums exist, each channel's RANK is
  computed directly: broadcast the [1, 256] sum row to all partitions with a
  K=1 PE matmul, is_gt against the per-partition sum, reduce -> rank[c].
  Selected channels (rank < 16) are then written straight from SBUF to HBM
  with ONE indirect scatter per half: dest sub-row = rank*8 + u, non-selected
  channels get an out-of-bounds dest and are silently skipped
  (bounds_check + oob_is_err=False).  No max8 chain, no index inversion, no
  gather re-read, no SBUF->HBM staging round trip on the critical path.
"""

import sys

if "/opt/trn_rl_repo" not in sys.path:
    sys.path.insert(0, "/opt/trn_rl_repo")

import numpy as np

from concourse import bacc, mybir, tile
from concourse.bass import IndirectOffsetOnAxis
from concourse.bass_utils import run_bass_kernel_spmd
from concourse.masks import make_identity

N_CORES = 8
B, C, H, W = 16, 256, 128, 128
K = 16
BPC = B // N_CORES          # samples per core = 2
E = H * W                   # 16384 elems per channel
GR = 8                      # gather sub-rows per channel (8 x 8KiB)
GP = K * GR                 # gather tile partitions
GW = E // GR                # gather row width (elems)
LOG2GR = GR.bit_length() - 1
ROWS = BPC * C              # 512 channel rows per core

f32 = mybir.dt.float32
i32 = mybir.dt.int32
u32 = mybir.dt.uint32
X = mybir.AxisListType.X
Alu = mybir.AluOpType

CHUNKS = [2048] * 8
# sample 1's last half tapers so the final reduce on the critical path is tiny
CHUNKS_S1H1 = [2048] * 5 + [1024] * 5 + [512] + [256] * 2

_cache = {}


def _build():
    nc = bacc.Bacc("TRN2", target_bir_lowering=False, debug=False,
                   num_devices=N_CORES)
    x_d = nc.dram_tensor("x", [ROWS, E], f32, kind="ExternalInput")
    y0_d = nc.dram_tensor("y0", [K * GR, GW], f32, kind="ExternalOutput")
    y1_d = nc.dram_tensor("y1", [K * GR, GW], f32, kind="ExternalOutput")

    with tile.TileContext(nc) as tc:
        with (
            tc.tile_pool(name="load", bufs=6) as load_pool,
            tc.tile_pool(name="small", bufs=1) as small,
            tc.tile_pool(name="gather", bufs=1) as gather_pool,
            tc.tile_pool(name="psum", bufs=1, space="PSUM") as psum,
        ):
            # ---- constants (no deps; scheduler fills gaps with these) ----
            ident = small.tile([128, 128], f32)
            make_identity(nc, ident[:])

            e_i = small.tile([K, GP], i32)
            nc.gpsimd.iota(out=e_i[:], pattern=[[1, GP]], base=0,
                           channel_multiplier=0)
            nc.vector.tensor_scalar(out=e_i[:], in0=e_i[:], scalar1=LOG2GR,
                                    scalar2=None, op0=Alu.arith_shift_right)
            e_f = small.tile([K, GP], f32)
            nc.vector.tensor_copy(e_f[:], e_i[:])
            col_i = small.tile([K, 1], i32)
            nc.gpsimd.iota(out=col_i[:], pattern=[[1, 1]], base=0,
                           channel_multiplier=1)
            col_f = small.tile([K, 1], f32)
            nc.vector.tensor_copy(col_f[:], col_i[:])
            e_mat = small.tile([K, GP], f32)
            nc.vector.tensor_scalar(out=e_mat[:], in0=e_f[:], scalar1=col_f[:],
                                    scalar2=None, op0=Alu.is_equal)

            pp = small.tile([GP, 1], i32)
            nc.gpsimd.iota(out=pp[:], pattern=[[1, 1]], base=0,
                           channel_multiplier=1)
            nc.vector.tensor_scalar(out=pp[:], in0=pp[:], scalar1=GR - 1,
                                    scalar2=None, op0=Alu.bitwise_and)
            a7f = small.tile([GP, 1], f32)
            nc.vector.tensor_copy(a7f[:], pp[:])

            # constants for the sample-1 rank/scatter path
            iota8_i = small.tile([128, GR], i32)
            nc.gpsimd.iota(out=iota8_i[:], pattern=[[1, GR]], base=0,
                           channel_multiplier=0)
            iota8_f = small.tile([128, GR], f32)
            nc.vector.tensor_copy(iota8_f[:], iota8_i[:])
            ones1 = small.tile([1, 128], f32)
            nc.vector.memset(ones1[:], 1.0)

            xg = x_d[:].rearrange("r (u e) -> (r u) e", u=GR)
            dma_engines = [nc.sync, nc.scalar]
            n_dma = 0

            # ================= sample 0: stream + max8 + gather =============
            s = 0
            ncols = len(CHUNKS)
            partials0 = small.tile([128, 2 * ncols], f32, tag="partials0")
            sums0 = small.tile([128, 2], f32, tag="sums0")
            psum_w0 = psum.tile([1, C], f32, tag="psw0")
            w_neg0 = small.tile([1, C], f32, tag="wneg0")
            for h in range(2):
                base = h * 128
                off = 0
                for j, cw in enumerate(CHUNKS):
                    t = load_pool.tile([128, 2048], f32)
                    eng = dma_engines[n_dma % 2]
                    n_dma += 1
                    eng.dma_start(out=t[:, 0:cw],
                                  in_=x_d[base:base + 128, off:off + cw])
                    nc.vector.reduce_sum(
                        out=partials0[:, h * ncols + j:h * ncols + j + 1],
                        in_=t[:, 0:cw], axis=X)
                    off += cw

                nc.vector.reduce_sum(
                    out=sums0[:, h:h + 1],
                    in_=partials0[:, h * ncols:h * ncols + len(CHUNKS)],
                    axis=X, negate=True)
                nc.tensor.matmul(out=psum_w0[:, h * 128:(h + 1) * 128],
                                 lhsT=sums0[:, h:h + 1], rhs=ident[:],
                                 start=True, stop=True)
                nc.vector.tensor_copy(w_neg0[:, h * 128:(h + 1) * 128],
                                      psum_w0[:, h * 128:(h + 1) * 128])

            # bottom-16 via two rounds of max8 on -sums; ranks 0-7
            # gather+store as soon as round 1's indices land
            m1 = small.tile([1, 8], f32, tag="m1")
            m2 = small.tile([1, 8], f32, tag="m2")
            idx_u = small.tile([1, K], u32, tag="idxu")
            w_rep = small.tile([1, C], f32, tag="wrep")
            half = GP // 2
            st_eng = [nc.sync, nc.scalar]
            g = gather_pool.tile([GP, GW], f32, tag="g0")

            def expand_and_gather(r, m):
                # gather-row index for tile partition p (p in [0, 64)):
                # idx[8r + (p>>3)]*GR + (p & (GR-1))
                idx_f = small.tile([1, 8], f32, tag=f"idxf{r}")
                nc.vector.tensor_copy(idx_f[:], idx_u[:, 8 * r:8 * r + 8])
                psum_t = psum.tile([8, 1], f32, tag="pst")
                nc.tensor.matmul(out=psum_t[:], lhsT=idx_f[:],
                                 rhs=ident[0:1, 0:1], start=True,
                                 stop=True)
                idx_t = small.tile([8, 1], f32, tag=f"idxt{r}")
                nc.vector.tensor_copy(idx_t[:], psum_t[:])
                psum_e = psum.tile([half, 1], f32, tag="pse")
                nc.tensor.matmul(out=psum_e[:], lhsT=e_mat[0:8, 0:half],
                                 rhs=idx_t[:], start=True, stop=True)
                idx64_f = small.tile([half, 1], f32, tag=f"i64f{r}")
                nc.vector.tensor_scalar(out=idx64_f[:], in0=psum_e[:],
                                        scalar1=float(GR),
                                        scalar2=0.0,
                                        op0=Alu.mult, op1=Alu.add)
                idx64_i = small.tile([half, 1], i32, tag=f"i64i{r}")
                nc.vector.tensor_tensor(out=idx64_i[:], in0=idx64_f[:],
                                        in1=a7f[0:half, :], op=Alu.add)
                # round 0 lands on partitions 0-63 (SDMA engines 0-7),
                # round 1 on 64-127 (engines 8-15)
                nc.gpsimd.indirect_dma_start(
                    out=g[r * half:(r + 1) * half, :], out_offset=None,
                    in_=xg,
                    in_offset=IndirectOffsetOnAxis(ap=idx64_i[:], axis=0))
                st_eng[r].dma_start(
                    out=y0_d[r * half:(r + 1) * half, :],
                    in_=g[r * half:(r + 1) * half, :])

            nc.vector.max(out=m1[:], in_=w_neg0[:])
            nc.vector.max_index(out=idx_u[:, 0:8], in_max=m1[:],
                                in_values=w_neg0[:])
            expand_and_gather(0, m1)
            nc.vector.match_replace(out=w_rep[:], in_to_replace=m1[:],
                                    in_values=w_neg0[:], imm_value=-1e38)
            nc.vector.max(out=m2[:], in_=w_rep[:])
            nc.vector.max_index(out=idx_u[:, 8:16], in_max=m2[:],
                                in_values=w_rep[:])
            expand_and_gather(1, m2)

            # ================= sample 1: resident + rank + scatter ==========
            res = small.tile([128, 2 * E], f32, tag="res")
            nc1 = len(CHUNKS_S1H1)
            partials1 = small.tile([128, 2 * nc1], f32, tag="partials1")
            sums1 = small.tile([128, 2], f32, tag="sums1")
            psum_w1 = psum.tile([1, C], f32, tag="psw1")
            w_row1 = small.tile([1, C], f32, tag="wrow1")
            for h in range(2):
                base = C + h * 128
                cl = CHUNKS if h == 0 else CHUNKS_S1H1
                off = 0
                for j, cw in enumerate(cl):
                    eng = dma_engines[n_dma % 2]
                    n_dma += 1
                    eng.dma_start(out=res[:, h * E + off:h * E + off + cw],
                                  in_=x_d[base:base + 128, off:off + cw])
                    nc.vector.reduce_sum(
                        out=partials1[:, h * nc1 + j:h * nc1 + j + 1],
                        in_=res[:, h * E + off:h * E + off + cw], axis=X)
                    off += cw

                nc.vector.reduce_sum(
                    out=sums1[:, h:h + 1],
                    in_=partials1[:, h * nc1:h * nc1 + len(cl)],
                    axis=X, negate=True)
                nc.tensor.matmul(out=psum_w1[:, h * 128:(h + 1) * 128],
                                 lhsT=sums1[:, h:h + 1], rhs=ident[:],
                                 start=True, stop=True)
                nc.vector.tensor_copy(w_row1[:, h * 128:(h + 1) * 128],
                                      psum_w1[:, h * 128:(h + 1) * 128])

            # broadcast the [1,256] row of -sums to every partition (K=1 mm)
            psum_b = psum.tile([128, C], f32, tag="psb")
            nc.tensor.matmul(out=psum_b[:], lhsT=ones1[:], rhs=w_row1[:],
                             start=True, stop=True)

            for gh in range(2):
                # rank of channel c (partition c of half gh) among all 256:
                # number of channels with a strictly smaller mean
                cmp_t = small.tile([128, C], f32, tag=f"cmp{gh}")
                nc.vector.tensor_scalar(out=cmp_t[:], in0=psum_b[:],
                                        scalar1=sums1[:, gh:gh + 1],
                                        scalar2=None, op0=Alu.is_gt)
                rank = small.tile([128, 1], f32, tag=f"rank{gh}")
                nc.vector.reduce_sum(out=rank[:], in_=cmp_t[:], axis=X)
                # dest sub-row = rank*8 + u if rank < 16, else OOB (skipped)
                r8 = small.tile([128, 1], f32, tag=f"r8_{gh}")
                nc.vector.tensor_scalar(out=r8[:], in0=rank[:], scalar1=8.0,
                                        scalar2=None, op0=Alu.mult)
                pen = small.tile([128, 1], f32, tag=f"pen{gh}")
                nc.vector.tensor_scalar(out=pen[:], in0=rank[:], scalar1=15.5,
                                        scalar2=1.0e6, op0=Alu.is_gt,
                                        op1=Alu.mult)
                r8p = small.tile([128, 1], f32, tag=f"r8p{gh}")
                nc.vector.tensor_tensor(out=r8p[:], in0=r8[:], in1=pen[:],
                                        op=Alu.add)
                offs_f = small.tile([128, GR], f32, tag=f"offf{gh}")
                nc.vector.tensor_scalar(out=offs_f[:], in0=iota8_f[:],
                                        scalar1=r8p[:], scalar2=None,
                                        op0=Alu.add)
                offs_i = small.tile([128, GR], i32, tag=f"offi{gh}")
                nc.vector.tensor_copy(offs_i[:], offs_f[:])

                # NOTE: in_ must stay a flat 2D [128, E] AP — a 3D
                # [128, 8, 2048] view of the same bytes fails on HW.  The
                # offset count (128x8) vs in_ size still pairs offsets[p, u]
                # with res[p, u*2048:(u+1)*2048] of this half.
                nc.gpsimd.indirect_dma_start(
                    out=y1_d[:],
                    out_offset=IndirectOffsetOnAxis(ap=offs_i[:], axis=0),
                    in_=res[:, gh * E:(gh + 1) * E], in_offset=None,
                    bounds_check=K * GR - 1, oob_is_err=False)

    nc.compile()
    return nc


def get_nc():
    if "nc" not in _cache:
        _cache["nc"] = _build()
    return _cache["nc"]


def make_in_maps(x: np.ndarray) -> list[dict[str, np.ndarray]]:
    x = np.ascontiguousarray(np.asarray(x, dtype=np.float32))
    assert x.shape == (B, C, H, W)
    return [{"x": x[c * BPC:(c + 1) * BPC].reshape(ROWS, E)}
            for c in range(N_CORES)]


def assemble(results: list[dict[str, np.ndarray]]) -> np.ndarray:
    out = np.empty((B, K, H, W), dtype=np.float32)
    for c in range(N_CORES):
        out[c * BPC] = results[c]["y0"].reshape(K, H, W)
        out[c * BPC + 1] = results[c]["y1"].reshape(K, H, W)
    return out


def kernel(x: np.ndarray) -> np.ndarray:
    nc = get_nc()
    res = run_bass_kernel_spmd(nc, make_in_maps(x), list(range(N_CORES)))
    return assemble(res.results)


# revision 5
# speedup vs baseline: 1.0399x; 1.0399x over previous
"""Bass/Trainium2 kernel for nn_Channel_attention (bottom-16 channel gather).

reference semantics (per sample b):
    weight = mean(x[b], axis=(H, W))           # [C]
    idx    = argsort(weight)[:16]              # ascending pooled value
    out[b] = x[b, idx]                         # [16, H, W]

Strategy: pure data parallel, B=16 sharded 2 samples per core over 8 cores.
Per core (x shard viewed as [512, 16384] = [(sample, channel), H*W]):

Sample 0 (fully overlapped under sample 1's loads):
  Stream [128ch, 2048] tiles through a small pool, DVE reduce -> per-channel
  sums, two rounds of max8/max_index/match_replace -> bottom-16 indices,
  SWDGE indirect-gather the selected channels from HBM, store to y0.

Sample 1 (the critical tail):
  Loads land in a persistent 16 MiB SBUF tile (128 part x 2 halves x 16 KiB),
  reduced as they arrive.  Once all 256 sums exist, each channel's RANK is
  computed directly: broadcast the [1, 256] sum row to all partitions with a
  K=1 PE matmul, is_gt against the per-partition sum, reduce -> rank[c].
  Selected channels (rank < 16) are then written straight from SBUF to HBM
  with ONE indirect scatter per half: dest sub-row = rank*8 + u, non-selected
  channels get an out-of-bounds dest and are silently skipped
  (bounds_check + oob_is_err=False).  No max8 chain, no index inversion, no
  gather re-read, no SBUF->HBM staging round trip on the critical path.
"""

import sys

if "/opt/trn_rl_repo" not in sys.path:
    sys.path.insert(0, "/opt/trn_rl_repo")

import numpy as np

from concourse import bacc, mybir, tile
from concourse.bass import IndirectOffsetOnAxis
from concourse.bass_utils import run_bass_kernel_spmd
from concourse.masks import make_identity

N_CORES = 8
B, C, H, W = 16, 256, 128, 128
K = 16
BPC = B // N_CORES          # samples per core = 2
E = H * W                   # 16384 elems per channel
GR = 8                      # gather sub-rows per channel (8 x 8KiB)
GP = K * GR                 # gather tile partitions
GW = E // GR                # gather row width (elems)
LOG2GR = GR.bit_length() - 1
ROWS = BPC * C              # 512 channel rows per core

f32 = mybir.dt.float32
i32 = mybir.dt.int32
u32 = mybir.dt.uint32
X = mybir.AxisListType.X
Alu = mybir.AluOpType

CHUNKS = [2048] * 8
# sample 1's last half tapers so the final reduce on the critical path is tiny
CHUNKS_S1H1 = [2048] * 5 + [1024] * 5 + [512] + [256] * 2

_cache = {}


def _build():
    nc = bacc.Bacc("TRN2", target_bir_lowering=False, debug=False,
                   num_devices=N_CORES)
    x_d = nc.dram_tensor("x", [ROWS, E], f32, kind="ExternalInput")
    y0_d = nc.dram_tensor("y0", [K * GR, GW], f32, kind="ExternalOutput")
    y1_d = nc.dram_tensor("y1", [K * GR, GW], f32, kind="ExternalOutput")

    with tile.TileContext(nc) as tc:
        with (
            tc.tile_pool(name="load", bufs=6) as load_pool,
            tc.tile_pool(name="small", bufs=1) as small,
            tc.tile_pool(name="gather", bufs=1) as gather_pool,
            tc.tile_pool(name="psum", bufs=1, space="PSUM") as psum,
        ):
            # ---- constants (no deps; scheduler fills gaps with these) ----
            ident = small.tile([128, 128], f32)
            make_identity(nc, ident[:])

            e_i = small.tile([K, GP], i32)
            nc.gpsimd.iota(out=e_i[:], pattern=[[1, GP]], base=0,
                           channel_multiplier=0)
            nc.vector.tensor_scalar(out=e_i[:], in0=e_i[:], scalar1=LOG2GR,
                                    scalar2=None, op0=Alu.arith_shift_right)
            e_f = small.tile([K, GP], f32)
            nc.vector.tensor_copy(e_f[:], e_i[:])
            col_i = small.tile([K, 1], i32)
            nc.gpsimd.iota(out=col_i[:], pattern=[[1, 1]], base=0,
                           channel_multiplier=1)
            col_f = small.tile([K, 1], f32)
            nc.vector.tensor_copy(col_f[:], col_i[:])
            e_mat = small.tile([K, GP], f32)
            nc.vector.tensor_scalar(out=e_mat[:], in0=e_f[:], scalar1=col_f[:],
                                    scalar2=None, op0=Alu.is_equal)

            pp = small.tile([GP, 1], i32)
            nc.gpsimd.iota(out=pp[:], pattern=[[1, 1]], base=0,
                           channel_multiplier=1)
            nc.vector.tensor_scalar(out=pp[:], in0=pp[:], scalar1=GR - 1,
                                    scalar2=None, op0=Alu.bitwise_and)
            a7f = small.tile([GP, 1], f32)
            nc.vector.tensor_copy(a7f[:], pp[:])

            # constants for the sample-1 rank/scatter path
            iota8_i = small.tile([128, GR], i32)
            nc.gpsimd.iota(out=iota8_i[:], pattern=[[1, GR]], base=0,
                           channel_multiplier=0)
            iota8_f = small.tile([128, GR], f32)
            nc.vector.tensor_copy(iota8_f[:], iota8_i[:])
            ones1 = small.tile([1, 128], f32)
            nc.vector.memset(ones1[:], 1.0)

            xg = x_d[:].rearrange("r (u e) -> (r u) e", u=GR)
            dma_engines = [nc.sync, nc.scalar]
            n_dma = 0

            # ================= sample 0: stream + max8 + gather =============
            s = 0
            ncols = len(CHUNKS)
            partials0 = small.tile([128, 2 * ncols], f32, tag="partials0")
            sums0 = small.tile([128, 2], f32, tag="sums0")
            psum_w0 = psum.tile([1, C], f32, tag="psw0")
            w_neg0 = small.tile([1, C], f32, tag="wneg0")
            s0_loads = []
            for h in range(2):
                base = h * 128
                off = 0
                for j, cw in enumerate(CHUNKS):
                    t = load_pool.tile([128, 2048], f32)
                    eng = dma_engines[n_dma % 2]
                    n_dma += 1
                    ld = eng.dma_start(out=t[:, 0:cw],
                                       in_=x_d[base:base + 128, off:off + cw])
                    s0_loads.append(ld)
                    nc.vector.reduce_sum(
                        out=partials0[:, h * ncols + j:h * ncols + j + 1],
                        in_=t[:, 0:cw], axis=X)
                    off += cw

                nc.vector.reduce_sum(
                    out=sums0[:, h:h + 1],
                    in_=partials0[:, h * ncols:h * ncols + len(CHUNKS)],
                    axis=X, negate=True)
                nc.tensor.matmul(out=psum_w0[:, h * 128:(h + 1) * 128],
                                 lhsT=sums0[:, h:h + 1], rhs=ident[:],
                                 start=True, stop=True)
                nc.vector.tensor_copy(w_neg0[:, h * 128:(h + 1) * 128],
                                      psum_w0[:, h * 128:(h + 1) * 128])

            # bottom-16 via two rounds of max8 on -sums; ranks 0-7
            # gather+store as soon as round 1's indices land
            m1 = small.tile([1, 8], f32, tag="m1")
            m2 = small.tile([1, 8], f32, tag="m2")
            idx_u = small.tile([1, K], u32, tag="idxu")
            w_rep = small.tile([1, C], f32, tag="wrep")
            half = GP // 2
            st_eng = [nc.sync, nc.scalar]
            g = gather_pool.tile([GP, GW], f32, tag="g0")

            def expand_and_gather(r, m):
                # gather-row index for tile partition p (p in [0, 64)):
                # idx[8r + (p>>3)]*GR + (p & (GR-1))
                idx_f = small.tile([1, 8], f32, tag=f"idxf{r}")
                nc.vector.tensor_copy(idx_f[:], idx_u[:, 8 * r:8 * r + 8])
                psum_t = psum.tile([8, 1], f32, tag="pst")
                nc.tensor.matmul(out=psum_t[:], lhsT=idx_f[:],
                                 rhs=ident[0:1, 0:1], start=True,
                                 stop=True)
                idx_t = small.tile([8, 1], f32, tag=f"idxt{r}")
                nc.vector.tensor_copy(idx_t[:], psum_t[:])
                psum_e = psum.tile([half, 1], f32, tag="pse")
                nc.tensor.matmul(out=psum_e[:], lhsT=e_mat[0:8, 0:half],
                                 rhs=idx_t[:], start=True, stop=True)
                idx64_f = small.tile([half, 1], f32, tag=f"i64f{r}")
                nc.vector.tensor_scalar(out=idx64_f[:], in0=psum_e[:],
                                        scalar1=float(GR),
                                        scalar2=0.0,
                                        op0=Alu.mult, op1=Alu.add)
                idx64_i = small.tile([half, 1], i32, tag=f"i64i{r}")
                nc.vector.tensor_tensor(out=idx64_i[:], in0=idx64_f[:],
                                        in1=a7f[0:half, :], op=Alu.add)
                # round 0 lands on partitions 0-63 (SDMA engines 0-7),
                # round 1 on 64-127 (engines 8-15)
                nc.gpsimd.indirect_dma_start(
                    out=g[r * half:(r + 1) * half, :], out_offset=None,
                    in_=xg,
                    in_offset=IndirectOffsetOnAxis(ap=idx64_i[:], axis=0))
                st_eng[r].dma_start(
                    out=y0_d[r * half:(r + 1) * half, :],
                    in_=g[r * half:(r + 1) * half, :])

            nc.vector.max(out=m1[:], in_=w_neg0[:])
            nc.vector.max_index(out=idx_u[:, 0:8], in_max=m1[:],
                                in_values=w_neg0[:])
            expand_and_gather(0, m1)
            nc.vector.match_replace(out=w_rep[:], in_to_replace=m1[:],
                                    in_values=w_neg0[:], imm_value=-1e38)
            nc.vector.max(out=m2[:], in_=w_rep[:])
            nc.vector.max_index(out=idx_u[:, 8:16], in_max=m2[:],
                                in_values=w_rep[:])
            expand_and_gather(1, m2)

            # ================= sample 1: resident + rank + scatter ==========
            res = small.tile([128, 2 * E], f32, tag="res")
            nc1 = len(CHUNKS_S1H1)
            partials1 = small.tile([128, 2 * nc1], f32, tag="partials1")
            sums1 = small.tile([128, 2], f32, tag="sums1")
            psum_w1 = psum.tile([1, C], f32, tag="psw1")
            w_row1 = small.tile([1, C], f32, tag="wrow1")
            n_s1 = 0
            for h in range(2):
                base = C + h * 128
                cl = CHUNKS if h == 0 else CHUNKS_S1H1
                off = 0
                for j, cw in enumerate(cl):
                    eng = dma_engines[n_dma % 2]
                    n_dma += 1
                    ld = eng.dma_start(
                        out=res[:, h * E + off:h * E + off + cw],
                        in_=x_d[base:base + 128, off:off + cw])
                    # resident-tile loads have no pool backpressure; without
                    # an explicit edge the scheduler interleaves them with
                    # sample 0's pool-paced loads and starves sample 0's
                    # pipeline (its gather+store chain then lands on the
                    # critical tail instead of hiding under these loads)
                    dep = s0_loads[min(n_s1, len(s0_loads) - 1)]
                    tile.add_dep_helper(
                        ld.ins, dep.ins,
                        reason="pace s1 resident load behind s0 stream")
                    n_s1 += 1
                    nc.vector.reduce_sum(
                        out=partials1[:, h * nc1 + j:h * nc1 + j + 1],
                        in_=res[:, h * E + off:h * E + off + cw], axis=X)
                    off += cw

                nc.vector.reduce_sum(
                    out=sums1[:, h:h + 1],
                    in_=partials1[:, h * nc1:h * nc1 + len(cl)],
                    axis=X, negate=True)
                nc.tensor.matmul(out=psum_w1[:, h * 128:(h + 1) * 128],
                                 lhsT=sums1[:, h:h + 1], rhs=ident[:],
                                 start=True, stop=True)
                nc.vector.tensor_copy(w_row1[:, h * 128:(h + 1) * 128],
                                      psum_w1[:, h * 128:(h + 1) * 128])

            # broadcast the [1,256] row of -sums to every partition (K=1 mm)
            psum_b = psum.tile([128, C], f32, tag="psb")
            nc.tensor.matmul(out=psum_b[:], lhsT=ones1[:], rhs=w_row1[:],
                             start=True, stop=True)

            for gh in range(2):
                # rank of channel c (partition c of half gh) among all 256:
                # number of channels with a strictly smaller mean
                cmp_t = small.tile([128, C], f32, tag=f"cmp{gh}")
                nc.vector.tensor_scalar(out=cmp_t[:], in0=psum_b[:],
                                        scalar1=sums1[:, gh:gh + 1],
                                        scalar2=None, op0=Alu.is_gt)
                rank = small.tile([128, 1], f32, tag=f"rank{gh}")
                nc.vector.reduce_sum(out=rank[:], in_=cmp_t[:], axis=X)
                # dest sub-row = rank*8 + u if rank < 16, else OOB (skipped)
                r8 = small.tile([128, 1], f32, tag=f"r8_{gh}")
                nc.vector.tensor_scalar(out=r8[:], in0=rank[:], scalar1=8.0,
                                        scalar2=None, op0=Alu.mult)
                pen = small.tile([128, 1], f32, tag=f"pen{gh}")
                nc.vector.tensor_scalar(out=pen[:], in0=rank[:], scalar1=15.5,
                                        scalar2=1.0e6, op0=Alu.is_gt,
                                        op1=Alu.mult)
                r8p = small.tile([128, 1], f32, tag=f"r8p{gh}")
                nc.vector.tensor_tensor(out=r8p[:], in0=r8[:], in1=pen[:],
                                        op=Alu.add)
                offs_f = small.tile([128, GR], f32, tag=f"offf{gh}")
                nc.vector.tensor_scalar(out=offs_f[:], in0=iota8_f[:],
                                        scalar1=r8p[:], scalar2=None,
                                        op0=Alu.add)
                offs_i = small.tile([128, GR], i32, tag=f"offi{gh}")
                nc.vector.tensor_copy(offs_i[:], offs_f[:])

                # NOTE: in_ must stay a flat 2D [128, E] AP — a 3D
                # [128, 8, 2048] view of the same bytes fails on HW.  The
                # offset count (128x8) vs in_ size still pairs offsets[p, u]
                # with res[p, u*2048:(u+1)*2048] of this half.
                nc.gpsimd.indirect_dma_start(
                    out=y1_d[:],
                    out_offset=IndirectOffsetOnAxis(ap=offs_i[:], axis=0),
                    in_=res[:, gh * E:(gh + 1) * E], in_offset=None,
                    bounds_check=K * GR - 1, oob_is_err=False)

    nc.compile()
    return nc


def get_nc():
    if "nc" not in _cache:
        _cache["nc"] = _build()
    return _cache["nc"]


def make_in_maps(x: np.ndarray) -> list[dict[str, np.ndarray]]:
    x = np.ascontiguousarray(np.asarray(x, dtype=np.float32))
    assert x.shape == (B, C, H, W)
    return [{"x": x[c * BPC:(c + 1) * BPC].reshape(ROWS, E)}
            for c in range(N_CORES)]


def assemble(results: list[dict[str, np.ndarray]]) -> np.ndarray:
    out = np.empty((B, K, H, W), dtype=np.float32)
    for c in range(N_CORES):
        out[c * BPC] = results[c]["y0"].reshape(K, H, W)
        out[c * BPC + 1] = results[c]["y1"].reshape(K, H, W)
    return out


def kernel(x: np.ndarray) -> np.ndarray:
    nc = get_nc()
    res = run_bass_kernel_spmd(nc, make_in_maps(x), list(range(N_CORES)))
    return assemble(res.results)


# revision 6
# speedup vs baseline: 1.1543x; 1.1100x over previous
"""Bass/Trainium2 kernel for nn_Channel_attention (bottom-16 channel gather).

reference semantics (per sample b):
    weight = mean(x[b], axis=(H, W))           # [C]
    idx    = argsort(weight)[:16]              # ascending pooled value
    out[b] = x[b, idx]                         # [16, H, W]

Strategy: pure data parallel, B=16 sharded 2 samples per core over 8 cores.
Per core (x shard viewed as [512, 16384] = [(sample, channel), H*W]):

Sample 0 (fully overlapped under sample 1's loads):
  Stream [128ch, 2048] tiles through a small pool, DVE reduce -> per-channel
  sums, two rounds of max8/max_index/match_replace -> bottom-16 indices,
  SWDGE indirect-gather the selected channels from HBM, store to y0.

Sample 1 (the critical tail):
  Loads land in a persistent 16 MiB SBUF tile (128 part x 2 halves x 16 KiB),
  reduced as they arrive.  Once all 256 sums exist, each channel's RANK is
  computed directly: broadcast the [1, 256] sum row to all partitions with a
  K=1 PE matmul, is_gt against the per-partition sum, reduce -> rank[c].
  Selected channels (rank < 16) are then written straight from SBUF to HBM
  with ONE indirect scatter per half: dest sub-row = rank*8 + u, non-selected
  channels get an out-of-bounds dest and are silently skipped
  (bounds_check + oob_is_err=False).  No max8 chain, no index inversion, no
  gather re-read, no SBUF->HBM staging round trip on the critical path.
"""

import sys

if "/opt/trn_rl_repo" not in sys.path:
    sys.path.insert(0, "/opt/trn_rl_repo")

import numpy as np

from concourse import bacc, mybir, tile
from concourse.bass import IndirectOffsetOnAxis
from concourse.bass_utils import run_bass_kernel_spmd
from concourse.masks import make_identity

N_CORES = 8
B, C, H, W = 16, 256, 128, 128
K = 16
BPC = B // N_CORES          # samples per core = 2
E = H * W                   # 16384 elems per channel
GR = 8                      # gather sub-rows per channel (8 x 8KiB)
GP = K * GR                 # gather tile partitions
GW = E // GR                # gather row width (elems)
LOG2GR = GR.bit_length() - 1
ROWS = BPC * C              # 512 channel rows per core

f32 = mybir.dt.float32
i32 = mybir.dt.int32
u32 = mybir.dt.uint32
X = mybir.AxisListType.X
Alu = mybir.AluOpType

CHUNKS = [2048] * 8
# sample 1's last half tapers so the final reduce on the critical path is tiny
CHUNKS_S1H1 = [2048] * 5 + [1024] * 5 + [512] + [256] * 2

_cache = {}


def _build():
    nc = bacc.Bacc("TRN2", target_bir_lowering=False, debug=False,
                   num_devices=N_CORES)
    x_d = nc.dram_tensor("x", [ROWS, E], f32, kind="ExternalInput")
    y0_d = nc.dram_tensor("y0", [K * GR, GW], f32, kind="ExternalOutput")
    y1_d = nc.dram_tensor("y1", [K * GR, GW], f32, kind="ExternalOutput")

    with tile.TileContext(nc) as tc:
        with (
            tc.tile_pool(name="load", bufs=6) as load_pool,
            tc.tile_pool(name="small", bufs=1) as small,
            tc.tile_pool(name="gather", bufs=1) as gather_pool,
            tc.tile_pool(name="psum", bufs=1, space="PSUM") as psum,
        ):
            # ---- constants (no deps; scheduler fills gaps with these) ----
            ident = small.tile([128, 128], f32)
            make_identity(nc, ident[:])

            e_i = small.tile([K, GP], i32)
            nc.gpsimd.iota(out=e_i[:], pattern=[[1, GP]], base=0,
                           channel_multiplier=0)
            nc.vector.tensor_scalar(out=e_i[:], in0=e_i[:], scalar1=LOG2GR,
                                    scalar2=None, op0=Alu.arith_shift_right)
            e_f = small.tile([K, GP], f32)
            nc.vector.tensor_copy(e_f[:], e_i[:])
            col_i = small.tile([K, 1], i32)
            nc.gpsimd.iota(out=col_i[:], pattern=[[1, 1]], base=0,
                           channel_multiplier=1)
            col_f = small.tile([K, 1], f32)
            nc.vector.tensor_copy(col_f[:], col_i[:])
            e_mat = small.tile([K, GP], f32)
            nc.vector.tensor_scalar(out=e_mat[:], in0=e_f[:], scalar1=col_f[:],
                                    scalar2=None, op0=Alu.is_equal)

            pp = small.tile([GP, 1], i32)
            nc.gpsimd.iota(out=pp[:], pattern=[[1, 1]], base=0,
                           channel_multiplier=1)
            nc.vector.tensor_scalar(out=pp[:], in0=pp[:], scalar1=GR - 1,
                                    scalar2=None, op0=Alu.bitwise_and)
            a7f = small.tile([GP, 1], f32)
            nc.vector.tensor_copy(a7f[:], pp[:])

            # constants for the sample-1 rank/scatter path
            iota8_i = small.tile([128, GR], i32)
            nc.gpsimd.iota(out=iota8_i[:], pattern=[[1, GR]], base=0,
                           channel_multiplier=0)
            iota8_f = small.tile([128, GR], f32)
            nc.vector.tensor_copy(iota8_f[:], iota8_i[:])
            ones1 = small.tile([1, 128], f32)
            nc.vector.memset(ones1[:], 1.0)

            xg = x_d[:].rearrange("r (u e) -> (r u) e", u=GR)
            dma_engines = [nc.sync, nc.scalar]
            n_dma = 0

            # ================= sample 0: stream + max8 + gather =============
            s = 0
            ncols = len(CHUNKS)
            partials0 = small.tile([128, 2 * ncols], f32, tag="partials0")
            sums0 = small.tile([128, 2], f32, tag="sums0")
            psum_w0 = psum.tile([1, C], f32, tag="psw0")
            w_neg0 = small.tile([1, C], f32, tag="wneg0")
            s0_loads = []
            for h in range(2):
                base = h * 128
                off = 0
                for j, cw in enumerate(CHUNKS):
                    t = load_pool.tile([128, 2048], f32)
                    eng = dma_engines[n_dma % 2]
                    n_dma += 1
                    ld = eng.dma_start(out=t[:, 0:cw],
                                       in_=x_d[base:base + 128, off:off + cw])
                    s0_loads.append(ld)
                    nc.vector.reduce_sum(
                        out=partials0[:, h * ncols + j:h * ncols + j + 1],
                        in_=t[:, 0:cw], axis=X)
                    off += cw

                nc.vector.reduce_sum(
                    out=sums0[:, h:h + 1],
                    in_=partials0[:, h * ncols:h * ncols + len(CHUNKS)],
                    axis=X, negate=True)
                nc.tensor.matmul(out=psum_w0[:, h * 128:(h + 1) * 128],
                                 lhsT=sums0[:, h:h + 1], rhs=ident[:],
                                 start=True, stop=True)
                nc.vector.tensor_copy(w_neg0[:, h * 128:(h + 1) * 128],
                                      psum_w0[:, h * 128:(h + 1) * 128])

            # bottom-16 via two rounds of max8 on -sums; ranks 0-7
            # gather+store as soon as round 1's indices land
            m1 = small.tile([1, 8], f32, tag="m1")
            m2 = small.tile([1, 8], f32, tag="m2")
            idx_u = small.tile([1, K], u32, tag="idxu")
            w_rep = small.tile([1, C], f32, tag="wrep")
            half = GP // 2
            st_eng = [nc.sync, nc.scalar]
            g = gather_pool.tile([GP, GW], f32, tag="g0")

            def expand_and_gather(r, m):
                # gather-row index for tile partition p (p in [0, 64)):
                # idx[8r + (p>>3)]*GR + (p & (GR-1))
                idx_f = small.tile([1, 8], f32, tag=f"idxf{r}")
                nc.vector.tensor_copy(idx_f[:], idx_u[:, 8 * r:8 * r + 8])
                psum_t = psum.tile([8, 1], f32, tag="pst")
                nc.tensor.matmul(out=psum_t[:], lhsT=idx_f[:],
                                 rhs=ident[0:1, 0:1], start=True,
                                 stop=True)
                idx_t = small.tile([8, 1], f32, tag=f"idxt{r}")
                nc.vector.tensor_copy(idx_t[:], psum_t[:])
                psum_e = psum.tile([half, 1], f32, tag="pse")
                nc.tensor.matmul(out=psum_e[:], lhsT=e_mat[0:8, 0:half],
                                 rhs=idx_t[:], start=True, stop=True)
                idx64_f = small.tile([half, 1], f32, tag=f"i64f{r}")
                nc.vector.tensor_scalar(out=idx64_f[:], in0=psum_e[:],
                                        scalar1=float(GR),
                                        scalar2=0.0,
                                        op0=Alu.mult, op1=Alu.add)
                idx64_i = small.tile([half, 1], i32, tag=f"i64i{r}")
                nc.vector.tensor_tensor(out=idx64_i[:], in0=idx64_f[:],
                                        in1=a7f[0:half, :], op=Alu.add)
                # round 0 lands on partitions 0-63 (SDMA engines 0-7),
                # round 1 on 64-127 (engines 8-15)
                nc.gpsimd.indirect_dma_start(
                    out=g[r * half:(r + 1) * half, :], out_offset=None,
                    in_=xg,
                    in_offset=IndirectOffsetOnAxis(ap=idx64_i[:], axis=0))
                st_eng[r].dma_start(
                    out=y0_d[r * half:(r + 1) * half, :],
                    in_=g[r * half:(r + 1) * half, :])

            nc.vector.max(out=m1[:], in_=w_neg0[:])
            nc.vector.max_index(out=idx_u[:, 0:8], in_max=m1[:],
                                in_values=w_neg0[:])
            expand_and_gather(0, m1)
            nc.vector.match_replace(out=w_rep[:], in_to_replace=m1[:],
                                    in_values=w_neg0[:], imm_value=-1e38)
            nc.vector.max(out=m2[:], in_=w_rep[:])
            nc.vector.max_index(out=idx_u[:, 8:16], in_max=m2[:],
                                in_values=w_rep[:])
            expand_and_gather(1, m2)

            # ================= sample 1: resident + rank + scatter ==========
            res = small.tile([128, 2 * E], f32, tag="res")
            nc1 = len(CHUNKS_S1H1)
            partials1 = small.tile([128, 2 * nc1], f32, tag="partials1")
            sums1 = small.tile([128, 2], f32, tag="sums1")
            psum_w1 = psum.tile([1, C], f32, tag="psw1")
            w_row1 = small.tile([1, C], f32, tag="wrow1")
            n_s1 = 0
            for h in range(2):
                base = C + h * 128
                cl = CHUNKS if h == 0 else CHUNKS_S1H1
                off = 0
                for j, cw in enumerate(cl):
                    eng = dma_engines[n_dma % 2]
                    n_dma += 1
                    ld = eng.dma_start(
                        out=res[:, h * E + off:h * E + off + cw],
                        in_=x_d[base:base + 128, off:off + cw])
                    # resident-tile loads have no pool backpressure; without
                    # an explicit edge the scheduler interleaves them with
                    # sample 0's pool-paced loads and starves sample 0's
                    # pipeline (its gather+store chain then lands on the
                    # critical tail instead of hiding under these loads).
                    # 2:1 ratio: sample 0 keeps bus priority and finishes
                    # ~2/3 through the stream, leaving its tail hidden.
                    dep = s0_loads[min(2 * n_s1 + 3, len(s0_loads) - 1)]
                    tile.add_dep_helper(
                        ld.ins, dep.ins,
                        reason="pace s1 resident load behind s0 stream")
                    n_s1 += 1
                    nc.vector.reduce_sum(
                        out=partials1[:, h * nc1 + j:h * nc1 + j + 1],
                        in_=res[:, h * E + off:h * E + off + cw], axis=X)
                    off += cw

                nc.vector.reduce_sum(
                    out=sums1[:, h:h + 1],
                    in_=partials1[:, h * nc1:h * nc1 + len(cl)],
                    axis=X, negate=True)
                nc.tensor.matmul(out=psum_w1[:, h * 128:(h + 1) * 128],
                                 lhsT=sums1[:, h:h + 1], rhs=ident[:],
                                 start=True, stop=True)
                nc.vector.tensor_copy(w_row1[:, h * 128:(h + 1) * 128],
                                      psum_w1[:, h * 128:(h + 1) * 128])

            # broadcast the [1,256] row of -sums to every partition (K=1 mm)
            psum_b = psum.tile([128, C], f32, tag="psb")
            nc.tensor.matmul(out=psum_b[:], lhsT=ones1[:], rhs=w_row1[:],
                             start=True, stop=True)

            for gh in range(2):
                # rank of channel c (partition c of half gh) among all 256:
                # number of channels with a strictly smaller mean
                cmp_t = small.tile([128, C], f32, tag=f"cmp{gh}")
                nc.vector.tensor_scalar(out=cmp_t[:], in0=psum_b[:],
                                        scalar1=sums1[:, gh:gh + 1],
                                        scalar2=None, op0=Alu.is_gt)
                rank = small.tile([128, 1], f32, tag=f"rank{gh}")
                nc.vector.reduce_sum(out=rank[:], in_=cmp_t[:], axis=X)
                # dest sub-row = rank*8 + u if rank < 16, else OOB (skipped)
                r8 = small.tile([128, 1], f32, tag=f"r8_{gh}")
                nc.vector.tensor_scalar(out=r8[:], in0=rank[:], scalar1=8.0,
                                        scalar2=None, op0=Alu.mult)
                pen = small.tile([128, 1], f32, tag=f"pen{gh}")
                nc.vector.tensor_scalar(out=pen[:], in0=rank[:], scalar1=15.5,
                                        scalar2=1.0e6, op0=Alu.is_gt,
                                        op1=Alu.mult)
                r8p = small.tile([128, 1], f32, tag=f"r8p{gh}")
                nc.vector.tensor_tensor(out=r8p[:], in0=r8[:], in1=pen[:],
                                        op=Alu.add)
                offs_f = small.tile([128, GR], f32, tag=f"offf{gh}")
                nc.vector.tensor_scalar(out=offs_f[:], in0=iota8_f[:],
                                        scalar1=r8p[:], scalar2=None,
                                        op0=Alu.add)
                offs_i = small.tile([128, GR], i32, tag=f"offi{gh}")
                nc.vector.tensor_copy(offs_i[:], offs_f[:])

                # NOTE: in_ must stay a flat 2D [128, E] AP — a 3D
                # [128, 8, 2048] view of the same bytes fails on HW.  The
                # offset count (128x8) vs in_ size still pairs offsets[p, u]
                # with res[p, u*2048:(u+1)*2048] of this half.
                nc.gpsimd.indirect_dma_start(
                    out=y1_d[:],
                    out_offset=IndirectOffsetOnAxis(ap=offs_i[:], axis=0),
                    in_=res[:, gh * E:(gh + 1) * E], in_offset=None,
                    bounds_check=K * GR - 1, oob_is_err=False)

    nc.compile()
    return nc


def get_nc():
    if "nc" not in _cache:
        _cache["nc"] = _build()
    return _cache["nc"]


def make_in_maps(x: np.ndarray) -> list[dict[str, np.ndarray]]:
    x = np.ascontiguousarray(np.asarray(x, dtype=np.float32))
    assert x.shape == (B, C, H, W)
    return [{"x": x[c * BPC:(c + 1) * BPC].reshape(ROWS, E)}
            for c in range(N_CORES)]


def assemble(results: list[dict[str, np.ndarray]]) -> np.ndarray:
    out = np.empty((B, K, H, W), dtype=np.float32)
    for c in range(N_CORES):
        out[c * BPC] = results[c]["y0"].reshape(K, H, W)
        out[c * BPC + 1] = results[c]["y1"].reshape(K, H, W)
    return out


def kernel(x: np.ndarray) -> np.ndarray:
    nc = get_nc()
    res = run_bass_kernel_spmd(nc, make_in_maps(x), list(range(N_CORES)))
    return assemble(res.results)


# revision 9
# speedup vs baseline: 1.1808x; 1.0230x over previous
"""Bass/Trainium2 kernel for nn_Channel_attention (bottom-16 channel gather).

reference semantics (per sample b):
    weight = mean(x[b], axis=(H, W))           # [C]
    idx    = argsort(weight)[:16]              # ascending pooled value
    out[b] = x[b, idx]                         # [16, H, W]

Strategy: pure data parallel, B=16 sharded 2 samples per core over 8 cores.
Per core (x shard viewed as [512, 16384] = [(sample, channel), H*W]):
  1. Stream [128ch, 2048] tiles, DVE reduce_add -> per-channel partial sums.
     Load DMAs alternate between the sync and scalar HWDGE queues.
  2. Per sample (pipelined so sample 0's tail hides under sample 1's loads):
     negate sums on DVE, PE-transpose into a [1, 256] row, two rounds of
     max8/max_index/match_replace -> bottom-16 channel indices in ascending
     order of pooled sum (argsort of sum == argsort of mean).
  3. Per max8 round, expand its 8 indices to 64 gather-row indices
     (idx*8 + subrow) with two tiny PE matmuls, SWDGE indirect-gather
     [64, 2048] (8 KiB rows are the line-rate descriptor size), and store
     contiguously; round 1's gather+store runs while round 2 still selects.
"""

import sys

if "/opt/trn_rl_repo" not in sys.path:
    sys.path.insert(0, "/opt/trn_rl_repo")

import numpy as np

from concourse import bacc, mybir, tile
from concourse.bass import IndirectOffsetOnAxis
from concourse.bass_utils import run_bass_kernel_spmd
from concourse.masks import make_identity

N_CORES = 8
B, C, H, W = 16, 256, 128, 128
K = 16
BPC = B // N_CORES          # samples per core = 2
E = H * W                   # 16384 elems per channel
GR = 8                      # gather sub-rows per channel (8 x 8KiB)
GP = K * GR                 # gather tile partitions
GW = E // GR                # gather row width (elems)
LOG2GR = GR.bit_length() - 1
ROWS = BPC * C              # 512 channel rows per core

f32 = mybir.dt.float32
i32 = mybir.dt.int32
u32 = mybir.dt.uint32
X = mybir.AxisListType.X
Alu = mybir.AluOpType

# chunk widths per (sample, half); last half of the last sample ends with
# small chunks so the final reduce exits quickly after the last load lands
CHUNKS = [2048] * 8
CHUNKS_LAST = [2048] * 5 + [1024] * 5 + [512] + [256] * 2

_cache = {}


def _build():
    nc = bacc.Bacc("TRN2", target_bir_lowering=False, debug=False,
                   num_devices=N_CORES)
    x_d = nc.dram_tensor("x", [ROWS, E], f32, kind="ExternalInput")
    y_d = nc.dram_tensor("y", [BPC * K * GR, GW], f32,
                         kind="ExternalOutput")

    with tile.TileContext(nc) as tc:
        with (
            tc.tile_pool(name="load", bufs=20) as load_pool,
            tc.tile_pool(name="small", bufs=1) as small,
            tc.tile_pool(name="gather", bufs=1) as gather_pool,
            tc.tile_pool(name="psum", bufs=1, space="PSUM") as psum,
        ):
            # ---- constants (no deps; scheduler fills gaps with these) ----
            ident = small.tile([128, 128], f32)
            make_identity(nc, ident[:])

            e_i = small.tile([K, GP], i32)
            nc.gpsimd.iota(out=e_i[:], pattern=[[1, GP]], base=0,
                           channel_multiplier=0)
            nc.vector.tensor_scalar(out=e_i[:], in0=e_i[:], scalar1=LOG2GR,
                                    scalar2=None, op0=Alu.arith_shift_right)
            e_f = small.tile([K, GP], f32)
            nc.vector.tensor_copy(e_f[:], e_i[:])
            col_i = small.tile([K, 1], i32)
            nc.gpsimd.iota(out=col_i[:], pattern=[[1, 1]], base=0,
                           channel_multiplier=1)
            col_f = small.tile([K, 1], f32)
            nc.vector.tensor_copy(col_f[:], col_i[:])
            e_mat = small.tile([K, GP], f32)
            nc.vector.tensor_scalar(out=e_mat[:], in0=e_f[:], scalar1=col_f[:],
                                    scalar2=None, op0=Alu.is_equal)

            pp = small.tile([GP, 1], i32)
            nc.gpsimd.iota(out=pp[:], pattern=[[1, 1]], base=0,
                           channel_multiplier=1)
            nc.vector.tensor_scalar(out=pp[:], in0=pp[:], scalar1=GR - 1,
                                    scalar2=None, op0=Alu.bitwise_and)
            a7f = small.tile([GP, 1], f32)
            nc.vector.tensor_copy(a7f[:], pp[:])

            xg = x_d[:].rearrange("r (u e) -> (r u) e", u=GR)
            dma_engines = [nc.sync, nc.scalar]
            n_dma = 0

            # ---- per-sample pipeline ----
            for s in range(BPC):
                ncols = 0
                chunk_lists = []
                for h in range(2):
                    cl = CHUNKS_LAST if (s == BPC - 1 and h == 1) else CHUNKS
                    chunk_lists.append(cl)
                    ncols = max(ncols, len(cl))
                partials = small.tile([128, 2 * ncols], f32, tag=f"partials{s}")

                sums = small.tile([128, 2], f32, tag=f"sums{s}")
                psum_w = psum.tile([1, C], f32, tag=f"psw{s}")
                w_neg = small.tile([1, C], f32, tag=f"wneg{s}")
                for h in range(2):
                    base = s * C + h * 128
                    off = 0
                    cl = chunk_lists[h]
                    for j, cw in enumerate(cl):
                        t = load_pool.tile([128, 2048], f32)
                        eng = dma_engines[n_dma % 2]
                        n_dma += 1
                        eng.dma_start(out=t[:, 0:cw],
                                      in_=x_d[base:base + 128, off:off + cw])
                        nc.vector.reduce_sum(
                            out=partials[:, h * ncols + j:h * ncols + j + 1],
                            in_=t[:, 0:cw], axis=X)
                        off += cw

                    # this half's sums + transpose, while the other half
                    # (or the next sample) is still streaming
                    nc.vector.reduce_sum(
                        out=sums[:, h:h + 1],
                        in_=partials[:, h * ncols:h * ncols + len(cl)],
                        axis=X, negate=True)
                    nc.tensor.matmul(out=psum_w[:, h * 128:(h + 1) * 128],
                                     lhsT=sums[:, h:h + 1], rhs=ident[:],
                                     start=True, stop=True)
                    nc.vector.tensor_copy(w_neg[:, h * 128:(h + 1) * 128],
                                          psum_w[:, h * 128:(h + 1) * 128])

                # bottom-16 via two rounds of max8 on -sums; ranks 0-7
                # gather+store as soon as round 1's indices land, while
                # round 2 is still running on DVE
                m1 = small.tile([1, 8], f32, tag=f"m1_{s}")
                m2 = small.tile([1, 8], f32, tag=f"m2_{s}")
                idx_u = small.tile([1, K], u32, tag=f"idxu{s}")
                w_rep = small.tile([1, C], f32, tag=f"wrep{s}")
                half = GP // 2
                st_eng = [nc.sync, nc.scalar]
                g = gather_pool.tile([GP, GW], f32, tag=f"g{s}")

                def expand_and_gather(r, m):
                    # gather-row index for tile partition p (p in [0, 64)):
                    # (s*C + idx[8r + (p>>3)])*GR + (p & (GR-1))
                    idx_f = small.tile([1, 8], f32, tag=f"idxf{s}_{r}")
                    nc.vector.tensor_copy(idx_f[:], idx_u[:, 8 * r:8 * r + 8])
                    psum_t = psum.tile([8, 1], f32, tag=f"pst{s}")
                    nc.tensor.matmul(out=psum_t[:], lhsT=idx_f[:],
                                     rhs=ident[0:1, 0:1], start=True,
                                     stop=True)
                    idx_t = small.tile([8, 1], f32, tag=f"idxt{s}_{r}")
                    nc.vector.tensor_copy(idx_t[:], psum_t[:])
                    psum_e = psum.tile([half, 1], f32, tag=f"pse{s}")
                    nc.tensor.matmul(out=psum_e[:], lhsT=e_mat[0:8, 0:half],
                                     rhs=idx_t[:], start=True, stop=True)
                    idx64_f = small.tile([half, 1], f32, tag=f"i64f{s}_{r}")
                    nc.vector.tensor_scalar(out=idx64_f[:], in0=psum_e[:],
                                            scalar1=float(GR),
                                            scalar2=float(s * C * GR),
                                            op0=Alu.mult, op1=Alu.add)
                    idx64_i = small.tile([half, 1], i32, tag=f"i64i{s}_{r}")
                    nc.vector.tensor_tensor(out=idx64_i[:], in0=idx64_f[:],
                                            in1=a7f[0:half, :], op=Alu.add)
                    # round 0 lands on partitions 0-63 (SDMA engines 0-7),
                    # round 1 on 64-127 (engines 8-15) so the two gathers'
                    # partition-bound descriptors run on disjoint engines
                    nc.gpsimd.indirect_dma_start(
                        out=g[r * half:(r + 1) * half, :], out_offset=None,
                        in_=xg,
                        in_offset=IndirectOffsetOnAxis(ap=idx64_i[:], axis=0))
                    st_eng[r].dma_start(
                        out=y_d[s * GP + r * half:s * GP + (r + 1) * half, :],
                        in_=g[r * half:(r + 1) * half, :])

                nc.vector.max(out=m1[:], in_=w_neg[:])
                nc.vector.max_index(out=idx_u[:, 0:8], in_max=m1[:],
                                    in_values=w_neg[:])
                expand_and_gather(0, m1)
                nc.vector.match_replace(out=w_rep[:], in_to_replace=m1[:],
                                        in_values=w_neg[:], imm_value=-1e38)
                nc.vector.max(out=m2[:], in_=w_rep[:])
                nc.vector.max_index(out=idx_u[:, 8:16], in_max=m2[:],
                                    in_values=w_rep[:])
                expand_and_gather(1, m2)

    nc.compile()
    return nc


def get_nc():
    if "nc" not in _cache:
        _cache["nc"] = _build()
    return _cache["nc"]


def make_in_maps(x: np.ndarray) -> list[dict[str, np.ndarray]]:
    x = np.ascontiguousarray(np.asarray(x, dtype=np.float32))
    assert x.shape == (B, C, H, W)
    return [{"x": x[c * BPC:(c + 1) * BPC].reshape(ROWS, E)}
            for c in range(N_CORES)]


def assemble(results: list[dict[str, np.ndarray]]) -> np.ndarray:
    out = np.empty((B, K, H, W), dtype=np.float32)
    for c in range(N_CORES):
        out[c * BPC:(c + 1) * BPC] = results[c]["y"].reshape(BPC, K, H, W)
    return out


def kernel(x: np.ndarray) -> np.ndarray:
    nc = get_nc()
    res = run_bass_kernel_spmd(nc, make_in_maps(x), list(range(N_CORES)))
    return assemble(res.results)

